# revision 33
# baseline (speedup 1.0000x reference)
"""Causal self-attention (B=8, T=1024, D=2048, H=16) on 8 NeuronCores.

Data-parallel over the batch dim: core i handles batch element i end-to-end
(QKV proj -> causal attention -> out proj). No collectives.

Layout: everything runs on transposed activations. The host feeds x[b].T
([D, T]) in fp16; Q/K are produced d-major ([Dh, T]), V token-major, and the
output projection emits y.T which the host transposes back. All matmul
operands are fp16 (same PE rate as f32r but half the LdWeights/DMA/SBUF
traffic); PSUM accumulation is fp32.

Softmax skips the max-subtraction (scores are ~N(0,1); exp is safely in fp16
range). The denominator is built by summing exp tiles on the DVE and doing a
single ones-column matmul per q-chunk, instead of a PE reduction chain per
k-tile. Causal structure is exploited at q-chunk=256 granularity (k-tiles
0..2jc+1 per chunk), and per-head attention outputs stay resident in SBUF as
the rhs of the output projection (no DRAM round-trip).
"""

import math

import numpy as np

B, T, D = 8, 1024, 2048
H = 16
DH = D // H  # 128
NCT = D // 128  # 16 c-tiles
QC = 256  # q-chunk for causal attention
NQC = T // QC  # 4
SCALE = 1.0 / math.sqrt(DH)
N_CORES = 8

_CACHE = {}


def _build():
    import concourse.bacc as bacc
    import concourse.mybir as mybir
    import concourse.tile as tile

    f32 = mybir.dt.float32
    f16 = mybir.dt.float16
    Exp = mybir.ActivationFunctionType.Exp

    nc = bacc.Bacc(None, target_bir_lowering=False)

    xT = nc.declare_dram_parameter("xT", [D, T], f16, isOutput=False)
    w_qkv = nc.declare_dram_parameter("w_qkv", [D, 3 * D], f16, isOutput=False)
    b_qkv = nc.declare_dram_parameter("b_qkv", [3 * D], f32, isOutput=False)
    b_v = nc.declare_dram_parameter("b_v", [D], f16, isOutput=False)
    w_proj = nc.declare_dram_parameter("w_proj", [D, D], f16, isOutput=False)
    b_proj = nc.declare_dram_parameter("b_proj", [D], f32, isOutput=False)
    consts = nc.declare_dram_parameter("consts", [128, 2 * QC + 129], f16, isOutput=False)
    outT = nc.declare_dram_parameter("outT", [D, T], f32, isOutput=True)

    with tile.TileContext(nc) as tc:
        with (
            tc.tile_pool(name="xbig", bufs=1) as pool_xbig,
            tc.tile_pool(name="vbig", bufs=1) as pool_vbig,
            tc.tile_pool(name="aobig", bufs=1) as pool_aobig,
            tc.tile_pool(name="qk", bufs=4) as pool_qk,
            tc.tile_pool(name="e", bufs=4) as pool_e,
            tc.tile_pool(name="esum", bufs=2) as pool_esum,
            tc.tile_pool(name="w512", bufs=2) as pool_w512,
            tc.tile_pool(name="wbig", bufs=2) as pool_wbig,
            tc.tile_pool(name="wproj", bufs=2) as pool_wproj,
            tc.tile_pool(name="outp", bufs=2) as pool_out,
            tc.tile_pool(name="den", bufs=3) as pool_den,
            tc.tile_pool(name="misc", bufs=1) as pool_misc,
        ):
            # ---- load x.T resident: 16 tiles [128, 1024], one per c-tile.
            # fc=0 weight DMAs are interleaved ahead of each xT tile and xT
            # is loaded in halves so the first matmuls start early; constant
            # and bias setup is deferred behind the first two c-tiles. ----
            # x and the fc=0 V-weights stream over BOTH DMA-issue queues
            # (sync + scalar), alternating by c-tile, to double the issue
            # bandwidth during the startup ramp.
            xT_t = []
            w_fc0 = []
            for ct in range(NCT):
                eng = nc.sync if ct % 2 == 0 else nc.scalar
                w_t = pool_w512.tile(
                    [128, 512], f16, name="w_fc0", tag="w512", bufs=6
                )
                eng.dma_start(
                    w_t[:],
                    w_qkv[ct * 128 : (ct + 1) * 128, 2 * D : 2 * D + 512],
                )
                w_fc0.append(w_t)
                t_ = pool_xbig.tile(
                    [128, T], f16, name="xT_t", tag="xbig", bufs=NCT
                )
                for half in range(2):
                    eng.dma_start(
                        t_[:, half * 512 : (half + 1) * 512],
                        xT[
                            ct * 128 : (ct + 1) * 128,
                            half * 512 : (half + 1) * 512,
                        ],
                    )
                xT_t.append(t_)

                if ct == 1:
                    # ---- constants / biases (off the critical DMA path).
                    # Causal masks (keep where k <= q), the ones column and
                    # the ones row all arrive precomputed from the host. ----
                    cpack = pool_misc.tile([128, 2, QC], f16, tag="cpack")
                    nc.scalar.dma_start(cpack[:], consts[:, : 2 * QC].rearrange("p (n f) -> p n f", n=2))
                    masks = [cpack[:, 0, :], cpack[:, 1, :]]
                    ones_col = pool_misc.tile([128, 1], f16, tag="ones_col")
                    nc.scalar.dma_start(ones_col[:], consts[:, 2 * QC : 2 * QC + 1])
                    ones_blk_t = pool_misc.tile([128, 128], f16, tag="ones_blk")
                    nc.scalar.dma_start(ones_blk_t[:], consts[:, 2 * QC + 1 : 2 * QC + 129])
                    ones_blk = ones_blk_t[:]

                    # b_qkv as [128, 48]: column j = feature-tile j
                    bqkv_sb = pool_misc.tile([128, 3 * D // 128], f32, tag="bqkv")
                    nc.scalar.dma_start(
                        bqkv_sb[:], b_qkv[:].rearrange("(n p) -> p n", p=128)
                    )
                    bproj_sb = pool_misc.tile([128, D // 128], f32, tag="bproj")
                    nc.scalar.dma_start(
                        bproj_sb[:], b_proj[:].rearrange("(n p) -> p n", p=128)
                    )
                    # V-bias broadcast to all partitions once on gpsimd; the
                    # V PSUM->SBUF copy then fuses the add on the DVE
                    bv_row = pool_misc.tile([1, D], f16, tag="bv_row")
                    nc.scalar.dma_start(bv_row[:], b_v[:].rearrange("(o f) -> o f", o=1))
                    bv_bcast = pool_misc.tile([128, D], f16, tag="bv_bcast")
                    nc.gpsimd.partition_broadcast(bv_bcast[:], bv_row[:])

            pool_qa_cm = tc.tile_pool(name="qaps", bufs=2, space="PSUM")
            pool_qa = pool_qa_cm.__enter__()

            # ---- phase 1: V for all heads, token-major [128, 8, 2048].
            # Six PSUM banks (token tiles in a 6+2 sub-pass split, weights
            # stay resident across both) so phase 2's Q/K accumulators are
            # pre-allocated and never wait on the phase-1 drain. ----
            V_sb = pool_vbig.tile([128, T // 128, D], f16, tag="vbig")
            with tc.tile_pool(name="p1psum", bufs=6, space="PSUM") as pool_p1:
                for fc in range(D // 512):
                    if fc == 0:
                        w_ts = w_fc0
                    else:
                        w_ts = []
                        for ct in range(NCT):
                            w_t = pool_w512.tile(
                                [128, 512], f16, name="w_t", tag="w512", bufs=6
                            )
                            nc.sync.dma_start(
                                w_t[:],
                                w_qkv[
                                    ct * 128 : (ct + 1) * 128,
                                    2 * D + fc * 512 : 2 * D + (fc + 1) * 512,
                                ],
                            )
                            w_ts.append(w_t)
                    # token tiles 0-5 use the phase-1 banks; tiles 6-7
                    # borrow the (idle) phase-2 Q/K accumulator banks
                    ps_v = [
                        pool_p1.tile([128, 512], f32, name="vps", tag="vps")
                        for _ in range(6)
                    ] + [
                        pool_qa.tile([128, 512], f32, name="vps_qa", tag="qa")
                        for _ in range(2)
                    ]
                    for ct in range(NCT):
                        for tt in range(T // 128):
                            nc.tensor.matmul(
                                ps_v[tt][:],
                                xT_t[ct][:, tt * 128 : (tt + 1) * 128],
                                w_ts[ct][:],
                                start=(ct == 0),
                                stop=(ct == NCT - 1),
                            )
                    for tt in range(T // 128):
                        # fused += b_v during the PSUM->SBUF copy
                        nc.vector.tensor_add(
                            V_sb[:, tt, fc * 512 : (fc + 1) * 512],
                            ps_v[tt][:],
                            bv_bcast[:, fc * 512 : (fc + 1) * 512],
                        )

            # ---- phase 2: per-head attention. Q/K projections run as four
            # half-chains (Q-half then K-half, bias-adds overlapping the
            # next chain); attention q-chunks 0-1 only need the first halves
            # of Q and K, so they interleave between the half-chains and the
            # PE never waits on a DVE bias-add. w_proj is staged into SBUF
            # row-contiguous (one c-tile per head) so phase 3 starts with
            # all weights resident. ----
            def emit_qk_weights(h):
                w_hs = {}
                for s, base in (("q", 0), ("k", D)):
                    # strided DMAs for the [D, 128] column block,
                    # c-tile major, split in two for pipelining
                    w_halves = []
                    for hf in range(2):
                        w_t = pool_wbig.tile(
                            [128, NCT // 2, 128],
                            f16,
                            name="w_t",
                            tag="wbig",
                            bufs=4,
                        )
                        nc.sync.dma_start(
                            w_t[:],
                            w_qkv[
                                hf * (D // 2) : (hf + 1) * (D // 2),
                                base + h * 128 : base + (h + 1) * 128,
                            ].rearrange("(n p) f -> p n f", p=128),
                        )
                        w_halves.append(w_t)
                    w_hs[s] = w_halves
                return w_hs

            def emit_qk_half(h, s, w_halves, sb, jc):
                btile = h if s == "q" else NCT + h
                ps = pool_qa.tile([128, 512], f32, name="qkps", tag="qa")
                for ct in range(NCT):
                    nc.tensor.matmul(
                        ps[:],
                        w_halves[ct // 8][:, ct % 8, :],
                        xT_t[ct][:, jc * 512 : (jc + 1) * 512],
                        start=(ct == 0),
                        stop=(ct == NCT - 1),
                    )
                nc.vector.tensor_scalar_add(
                    sb[:, jc * 512 : (jc + 1) * 512],
                    ps[:],
                    bqkv_sb[:, btile : btile + 1],
                )

            def emit_attn_chunks(h, qk, ao_t, jcs):
                # causal attention, scores transposed [k, q],
                # q-chunks of 256 (k-tiles 0..2jc+1; rest masked)
                for jc in jcs:
                    nk = 2 * jc + 2
                    ps_y = pool_y.tile([128, QC], f32, tag="y")
                    e_sum = pool_esum.tile([128, QC], f16, tag="esum", bufs=3)
                    for ki in range(nk):
                        ps_s = pool_s.tile([128, QC], f32, tag="mm256")
                        nc.tensor.matmul(
                            ps_s[:],
                            qk["k"][:, ki * 128 : (ki + 1) * 128],
                            qk["q"][:, jc * QC : (jc + 1) * QC],
                            start=True,
                            stop=True,
                        )
                        # exp of the first k-tile lands directly in e_sum
                        e_t = (
                            e_sum
                            if ki == 0
                            else pool_e.tile([128, QC], f16, tag="e", bufs=4)
                        )
                        nc.scalar.activation(e_t[:], ps_s[:], Exp, scale=SCALE)
                        r = ki - 2 * jc
                        if r >= 0:
                            # causal mask for the diagonal k-tiles: one
                            # DVE multiply with a precomputed 0/1 tile
                            nc.vector.tensor_mul(e_t[:], e_t[:], masks[r])
                        nc.tensor.matmul(
                            ps_y[:],
                            V_sb[:, ki, h * 128 : (h + 1) * 128],
                            e_t[:],
                            start=(ki == 0),
                            stop=(ki == nk - 1),
                        )
                        if ki > 0:
                            nc.vector.tensor_add(e_sum[:], e_sum[:], e_t[:])
                    # one ones-matrix matmul yields the denominator already
                    # broadcast across partitions: out[m, q] = sum_k e_sum[k, q]
                    ps_db = pool_s.tile([128, QC], f32, name="ps_db", tag="mm256")
                    nc.tensor.matmul(
                        ps_db[:], ones_blk, e_sum[:], start=True, stop=True
                    )
                    # approx reciprocal (~18 bits; denominators are
                    # bounded away from 0 by the diagonal exp term)
                    inv_b = pool_den.tile([128, QC], f32, name="inv_b", tag="invb")
                    nc.vector.reciprocal_approx_fast(out=inv_b[:], in_=ps_db[:])
                    nc.vector.tensor_mul(
                        ao_t[:, jc * QC : (jc + 1) * QC], ps_y[:], inv_b[:]
                    )

            with (
                tc.tile_pool(name="sps", bufs=4, space="PSUM") as pool_s,
                tc.tile_pool(name="yps", bufs=2, space="PSUM") as pool_y,
            ):
                ao_heads = []
                wp_full = []
                for h in range(H):
                    # stage one row-contiguous c-tile of w_proj per head on
                    # the scalar DMA queue (ready before phase 3 starts)
                    wp_t = pool_wproj.tile(
                        [128, D], f16, name="wp_t", tag="wproj", bufs=NCT
                    )
                    nc.scalar.dma_start(
                        wp_t[:], w_proj[h * 128 : (h + 1) * 128, :]
                    )
                    wp_full.append(wp_t)

                    w_hs = emit_qk_weights(h)
                    q_sb = pool_qk.tile([128, T], f16, name="q_sb", tag="qk")
                    k_sb = pool_qk.tile([128, T], f16, name="k_sb", tag="qk")
                    qk = {"q": q_sb, "k": k_sb}
                    ao_t = pool_aobig.tile(
                        [128, T], f16, name="ao_t", tag="aobig", bufs=H
                    )
                    # chunks 0-1 read only the first halves of Q and K, so
                    # they run between the half-chains and hide the DVE
                    # bias-add latency
                    emit_qk_half(h, "q", w_hs["q"], q_sb, 0)
                    emit_qk_half(h, "k", w_hs["k"], k_sb, 0)
                    emit_qk_half(h, "q", w_hs["q"], q_sb, 1)
                    emit_attn_chunks(h, qk, ao_t, (0, 1))
                    emit_qk_half(h, "k", w_hs["k"], k_sb, 1)
                    emit_attn_chunks(h, qk, ao_t, (2, 3))
                    ao_heads.append(ao_t)

            # ---- phase 3: output projection, emitted transposed.
            # rhs for c-tile ct is exactly head ct's attention output
            # (f = h*128 + dh); weights and activations are all resident. ----
            with tc.tile_pool(name="p3psum", bufs=4, space="PSUM") as pool_p3:
                for dt in range(D // 128):
                    ps3 = [
                        pool_p3.tile([128, 512], f32, name="ps3", tag="mm512")
                        for _ in range(2)
                    ]
                    for ct in range(NCT):
                        for jc in range(2):
                            nc.tensor.matmul(
                                ps3[jc][:],
                                wp_full[ct][:, dt * 128 : (dt + 1) * 128],
                                ao_heads[ct][:, jc * 512 : (jc + 1) * 512],
                                start=(ct == 0),
                                stop=(ct == NCT - 1),
                            )
                    for jc in range(2):
                        o_t = pool_out.tile([128, 512], f32, tag="outp")
                        nc.vector.tensor_scalar_add(
                            o_t[:], ps3[jc][:], bproj_sb[:, dt : dt + 1]
                        )
                        nc.sync.dma_start(
                            outT[dt * 128 : (dt + 1) * 128, jc * 512 : (jc + 1) * 512],
                            o_t[:],
                        )

            pool_qa_cm.__exit__(None, None, None)

    nc.compile()
    return nc


def _get_nc():
    if "nc" not in _CACHE:
        _CACHE["nc"] = _build()
    return _CACHE["nc"]


def kernel(x, w_qkv, b_qkv, w_proj, b_proj, _trace=False, _trace_kwargs=None):
    from concourse.bass_utils import run_bass_kernel_spmd

    x = np.asarray(x, dtype=np.float32)
    w_qkv = np.asarray(w_qkv, dtype=np.float32)
    b_qkv = np.asarray(b_qkv, dtype=np.float32)
    w_proj = np.asarray(w_proj, dtype=np.float32)
    b_proj = np.asarray(b_proj, dtype=np.float32)

    w_qkv16 = np.ascontiguousarray(w_qkv.astype(np.float16))
    w_proj16 = np.ascontiguousarray(w_proj.astype(np.float16))
    b_v16 = np.ascontiguousarray(b_qkv[2 * D : 3 * D].astype(np.float16))

    # packed constants: causal masks (keep where k<=q) for the two diagonal
    # k-tiles, a ones column, and a ones row
    consts = np.zeros((128, 2 * QC + 129), dtype=np.float16)
    p = np.arange(128)[:, None]
    f = np.arange(QC)[None, :]
    consts[:, 0:QC] = f >= p
    consts[:, QC : 2 * QC] = f >= p + 128
    consts[:, 2 * QC : 2 * QC + 129] = 1.0

    nc = _get_nc()
    in_maps = []
    for i in range(N_CORES):
        in_maps.append(
            {
                "xT": np.ascontiguousarray(x[i].T.astype(np.float16)),
                "w_qkv": w_qkv16,
                "b_qkv": b_qkv,
                "b_v": b_v16,
                "w_proj": w_proj16,
                "b_proj": b_proj,
                "consts": consts,
            }
        )
    res = run_bass_kernel_spmd(
        nc,
        in_maps,
        list(range(N_CORES)),
        trace=_trace,
        **(_trace_kwargs or {}),
    )
    y = np.stack(
        [np.ascontiguousarray(res.results[i]["outT"].T) for i in range(N_CORES)]
    )
    if _trace:
        _CACHE["last_result"] = res
    return y


# revision 34
# speedup vs baseline: 1.0112x; 1.0112x over previous
"""Causal self-attention (B=8, T=1024, D=2048, H=16) on 8 NeuronCores.

Data-parallel over the batch dim: core i handles batch element i end-to-end
(QKV proj -> causal attention -> out proj). No collectives.

Layout: everything runs on transposed activations. The host feeds x[b].T
([D, T]) in fp16; Q/K are produced d-major ([Dh, T]), V token-major, and the
output projection emits y.T which the host transposes back. All matmul
operands are fp16 (same PE rate as f32r but half the LdWeights/DMA/SBUF
traffic); PSUM accumulation is fp32.

Softmax skips the max-subtraction (scores are ~N(0,1); exp is safely in fp16
range). Exp tiles are summed on the DVE and one ones-matrix matmul per
q-chunk then yields the softmax denominator already broadcast across
partitions (reciprocal + normalize run directly on the DVE). Causal
structure is exploited at q-chunk=256 granularity (k-tiles 0..2jc+1 per
chunk) with the mask applied as a DVE multiply against precomputed 0/1
tiles. Q/K projections run as four half-chains interleaved with the
attention chunks that consume them, hiding the PSUM->SBUF bias-add latency.
Per-head attention outputs and a row-contiguous copy of w_proj stay
resident in SBUF, so the output projection starts with everything on-chip
(no DRAM round-trip). x and weights stream over both DMA-issue queues
(sync + scalar) during the startup ramp.
"""

import math

import numpy as np

B, T, D = 8, 1024, 2048
H = 16
DH = D // H  # 128
NCT = D // 128  # 16 c-tiles
QC = 256  # q-chunk for causal attention
NQC = T // QC  # 4
SCALE = 1.0 / math.sqrt(DH)
N_CORES = 8

_CACHE = {}


def _build():
    import concourse.bacc as bacc
    import concourse.mybir as mybir
    import concourse.tile as tile

    f32 = mybir.dt.float32
    f16 = mybir.dt.float16
    Exp = mybir.ActivationFunctionType.Exp

    nc = bacc.Bacc(None, target_bir_lowering=False)

    xT = nc.declare_dram_parameter("xT", [D, T], f16, isOutput=False)
    w_qkv = nc.declare_dram_parameter("w_qkv", [D, 3 * D], f16, isOutput=False)
    b_qkv = nc.declare_dram_parameter("b_qkv", [3 * D], f32, isOutput=False)
    b_v = nc.declare_dram_parameter("b_v", [D], f16, isOutput=False)
    w_proj = nc.declare_dram_parameter("w_proj", [D, D], f16, isOutput=False)
    b_proj = nc.declare_dram_parameter("b_proj", [D], f32, isOutput=False)
    consts = nc.declare_dram_parameter("consts", [128, 2 * QC + 129], f16, isOutput=False)
    outT = nc.declare_dram_parameter("outT", [D, T], f32, isOutput=True)

    with tile.TileContext(nc) as tc:
        with (
            tc.tile_pool(name="xbig", bufs=1) as pool_xbig,
            tc.tile_pool(name="vbig", bufs=1) as pool_vbig,
            tc.tile_pool(name="aobig", bufs=1) as pool_aobig,
            tc.tile_pool(name="qk", bufs=4) as pool_qk,
            tc.tile_pool(name="e", bufs=4) as pool_e,
            tc.tile_pool(name="esum", bufs=2) as pool_esum,
            tc.tile_pool(name="w512", bufs=2) as pool_w512,
            tc.tile_pool(name="wbig", bufs=2) as pool_wbig,
            tc.tile_pool(name="wproj", bufs=2) as pool_wproj,
            tc.tile_pool(name="outp", bufs=2) as pool_out,
            tc.tile_pool(name="den", bufs=2) as pool_den,
            tc.tile_pool(name="misc", bufs=1) as pool_misc,
        ):
            # ---- load x.T resident: 16 tiles [128, 1024], one per c-tile.
            # fc=0 weight DMAs are interleaved ahead of each xT tile and xT
            # is loaded in halves so the first matmuls start early; constant
            # and bias setup is deferred behind the first two c-tiles. ----
            # x and the fc=0 V-weights stream over BOTH DMA-issue queues
            # (sync + scalar), alternating by c-tile, to double the issue
            # bandwidth during the startup ramp.
            xT_t = []
            w_fc0 = []
            for ct in range(NCT):
                eng = nc.sync if ct % 2 == 0 else nc.scalar
                w_t = pool_w512.tile(
                    [128, 512], f16, name="w_fc0", tag="w512", bufs=6
                )
                eng.dma_start(
                    w_t[:],
                    w_qkv[ct * 128 : (ct + 1) * 128, 2 * D : 2 * D + 512],
                )
                w_fc0.append(w_t)
                t_ = pool_xbig.tile(
                    [128, T], f16, name="xT_t", tag="xbig", bufs=NCT
                )
                for half in range(2):
                    eng.dma_start(
                        t_[:, half * 512 : (half + 1) * 512],
                        xT[
                            ct * 128 : (ct + 1) * 128,
                            half * 512 : (half + 1) * 512,
                        ],
                    )
                xT_t.append(t_)

                if ct == 1:
                    # ---- constants / biases (off the critical DMA path).
                    # Causal masks (keep where k <= q), the ones column and
                    # the ones row all arrive precomputed from the host. ----
                    cpack = pool_misc.tile([128, 2, QC], f16, tag="cpack")
                    nc.scalar.dma_start(cpack[:], consts[:, : 2 * QC].rearrange("p (n f) -> p n f", n=2))
                    masks = [cpack[:, 0, :], cpack[:, 1, :]]
                    ones_blk_t = pool_misc.tile([128, 128], f16, tag="ones_blk")
                    nc.scalar.dma_start(ones_blk_t[:], consts[:, 2 * QC + 1 : 2 * QC + 129])
                    ones_blk = ones_blk_t[:]

                    # b_qkv as [128, 48]: column j = feature-tile j
                    bqkv_sb = pool_misc.tile([128, 3 * D // 128], f32, tag="bqkv")
                    nc.scalar.dma_start(
                        bqkv_sb[:], b_qkv[:].rearrange("(n p) -> p n", p=128)
                    )
                    bproj_sb = pool_misc.tile([128, D // 128], f32, tag="bproj")
                    nc.scalar.dma_start(
                        bproj_sb[:], b_proj[:].rearrange("(n p) -> p n", p=128)
                    )
                    # V-bias broadcast to all partitions once on gpsimd; the
                    # V PSUM->SBUF copy then fuses the add on the DVE
                    bv_row = pool_misc.tile([1, D], f16, tag="bv_row")
                    nc.scalar.dma_start(bv_row[:], b_v[:].rearrange("(o f) -> o f", o=1))
                    bv_bcast = pool_misc.tile([128, D], f16, tag="bv_bcast")
                    nc.gpsimd.partition_broadcast(bv_bcast[:], bv_row[:])

            pool_qa_cm = tc.tile_pool(name="qaps", bufs=2, space="PSUM")
            pool_qa = pool_qa_cm.__enter__()

            # ---- phase 1: V for all heads, token-major [128, 8, 2048].
            # Six PSUM banks (token tiles in a 6+2 sub-pass split, weights
            # stay resident across both) so phase 2's Q/K accumulators are
            # pre-allocated and never wait on the phase-1 drain. ----
            V_sb = pool_vbig.tile([128, T // 128, D], f16, tag="vbig")
            with tc.tile_pool(name="p1psum", bufs=6, space="PSUM") as pool_p1:
                for fc in range(D // 512):
                    if fc == 0:
                        w_ts = w_fc0
                    else:
                        w_ts = []
                        for ct in range(NCT):
                            w_t = pool_w512.tile(
                                [128, 512], f16, name="w_t", tag="w512", bufs=6
                            )
                            nc.sync.dma_start(
                                w_t[:],
                                w_qkv[
                                    ct * 128 : (ct + 1) * 128,
                                    2 * D + fc * 512 : 2 * D + (fc + 1) * 512,
                                ],
                            )
                            w_ts.append(w_t)
                    # token tiles 0-5 use the phase-1 banks; tiles 6-7
                    # borrow the (idle) phase-2 Q/K accumulator banks
                    ps_v = [
                        pool_p1.tile([128, 512], f32, name="vps", tag="vps")
                        for _ in range(6)
                    ] + [
                        pool_qa.tile([128, 512], f32, name="vps_qa", tag="qa")
                        for _ in range(2)
                    ]
                    for ct in range(NCT):
                        for tt in range(T // 128):
                            nc.tensor.matmul(
                                ps_v[tt][:],
                                xT_t[ct][:, tt * 128 : (tt + 1) * 128],
                                w_ts[ct][:],
                                start=(ct == 0),
                                stop=(ct == NCT - 1),
                            )
                    for tt in range(T // 128):
                        # fused += b_v during the PSUM->SBUF copy
                        nc.vector.tensor_add(
                            V_sb[:, tt, fc * 512 : (fc + 1) * 512],
                            ps_v[tt][:],
                            bv_bcast[:, fc * 512 : (fc + 1) * 512],
                        )

            # ---- phase 2: per-head attention. Q/K projections run as four
            # half-chains (Q-half then K-half, bias-adds overlapping the
            # next chain); attention q-chunks 0-1 only need the first halves
            # of Q and K, so they interleave between the half-chains and the
            # PE never waits on a DVE bias-add. w_proj is staged into SBUF
            # row-contiguous (one c-tile per head) so phase 3 starts with
            # all weights resident. ----
            def emit_qk_weights(h):
                w_hs = {}
                for s, base in (("q", 0), ("k", D)):
                    # strided DMAs for the [D, 128] column block,
                    # c-tile major, split in two for pipelining
                    w_halves = []
                    for hf in range(2):
                        w_t = pool_wbig.tile(
                            [128, NCT // 2, 128],
                            f16,
                            name="w_t",
                            tag="wbig",
                            bufs=4,
                        )
                        nc.sync.dma_start(
                            w_t[:],
                            w_qkv[
                                hf * (D // 2) : (hf + 1) * (D // 2),
                                base + h * 128 : base + (h + 1) * 128,
                            ].rearrange("(n p) f -> p n f", p=128),
                        )
                        w_halves.append(w_t)
                    w_hs[s] = w_halves
                return w_hs

            def emit_qk_half(h, s, w_halves, sb, jc):
                btile = h if s == "q" else NCT + h
                ps = pool_qa.tile([128, 512], f32, name="qkps", tag="qa")
                for ct in range(NCT):
                    nc.tensor.matmul(
                        ps[:],
                        w_halves[ct // 8][:, ct % 8, :],
                        xT_t[ct][:, jc * 512 : (jc + 1) * 512],
                        start=(ct == 0),
                        stop=(ct == NCT - 1),
                    )
                nc.vector.tensor_scalar_add(
                    sb[:, jc * 512 : (jc + 1) * 512],
                    ps[:],
                    bqkv_sb[:, btile : btile + 1],
                )

            def emit_attn_chunks(h, qk, ao_t, jcs):
                # causal attention, scores transposed [k, q],
                # q-chunks of 256 (k-tiles 0..2jc+1; rest masked)
                for jc in jcs:
                    nk = 2 * jc + 2
                    ps_y = pool_y.tile([128, QC], f32, tag="y")
                    e_sum = pool_esum.tile([128, QC], f16, tag="esum", bufs=3)
                    for ki in range(nk):
                        ps_s = pool_s.tile([128, QC], f32, tag="mm256")
                        nc.tensor.matmul(
                            ps_s[:],
                            qk["k"][:, ki * 128 : (ki + 1) * 128],
                            qk["q"][:, jc * QC : (jc + 1) * QC],
                            start=True,
                            stop=True,
                        )
                        # exp of the first k-tile lands directly in e_sum
                        e_t = (
                            e_sum
                            if ki == 0
                            else pool_e.tile([128, QC], f16, tag="e", bufs=4)
                        )
                        nc.scalar.activation(e_t[:], ps_s[:], Exp, scale=SCALE)
                        r = ki - 2 * jc
                        if r >= 0:
                            # causal mask for the diagonal k-tiles: one
                            # DVE multiply with a precomputed 0/1 tile
                            nc.vector.tensor_mul(e_t[:], e_t[:], masks[r])
                        nc.tensor.matmul(
                            ps_y[:],
                            V_sb[:, ki, h * 128 : (h + 1) * 128],
                            e_t[:],
                            start=(ki == 0),
                            stop=(ki == nk - 1),
                        )
                        if ki > 0:
                            nc.vector.tensor_add(e_sum[:], e_sum[:], e_t[:])
                    # one ones-matrix matmul yields the denominator already
                    # broadcast across partitions: out[m, q] = sum_k e_sum[k, q]
                    ps_db = pool_s.tile([128, QC], f32, name="ps_db", tag="mm256")
                    nc.tensor.matmul(
                        ps_db[:], ones_blk, e_sum[:], start=True, stop=True
                    )
                    # approx reciprocal (~18 bits; denominators are
                    # bounded away from 0 by the diagonal exp term)
                    inv_b = pool_den.tile([128, QC], f32, name="inv_b", tag="invb")
                    nc.vector.reciprocal_approx_fast(out=inv_b[:], in_=ps_db[:])
                    nc.vector.tensor_mul(
                        ao_t[:, jc * QC : (jc + 1) * QC], ps_y[:], inv_b[:]
                    )

            with (
                tc.tile_pool(name="sps", bufs=4, space="PSUM") as pool_s,
                tc.tile_pool(name="yps", bufs=2, space="PSUM") as pool_y,
            ):
                ao_heads = []
                wp_full = []
                for h in range(H):
                    # stage one row-contiguous c-tile of w_proj per head on
                    # the scalar DMA queue (ready before phase 3 starts)
                    wp_t = pool_wproj.tile(
                        [128, D], f16, name="wp_t", tag="wproj", bufs=NCT
                    )
                    nc.scalar.dma_start(
                        wp_t[:], w_proj[h * 128 : (h + 1) * 128, :]
                    )
                    wp_full.append(wp_t)

                    w_hs = emit_qk_weights(h)
                    q_sb = pool_qk.tile([128, T], f16, name="q_sb", tag="qk")
                    k_sb = pool_qk.tile([128, T], f16, name="k_sb", tag="qk")
                    qk = {"q": q_sb, "k": k_sb}
                    ao_t = pool_aobig.tile(
                        [128, T], f16, name="ao_t", tag="aobig", bufs=H
                    )
                    # chunks 0-1 read only the first halves of Q and K, so
                    # they run between the half-chains and hide the DVE
                    # bias-add latency
                    emit_qk_half(h, "q", w_hs["q"], q_sb, 0)
                    emit_qk_half(h, "k", w_hs["k"], k_sb, 0)
                    emit_qk_half(h, "q", w_hs["q"], q_sb, 1)
                    emit_attn_chunks(h, qk, ao_t, (0, 1))
                    emit_qk_half(h, "k", w_hs["k"], k_sb, 1)
                    emit_attn_chunks(h, qk, ao_t, (2, 3))
                    ao_heads.append(ao_t)

            # ---- phase 3: output projection, emitted transposed.
            # rhs for c-tile ct is exactly head ct's attention output
            # (f = h*128 + dh); weights and activations are all resident. ----
            with tc.tile_pool(name="p3psum", bufs=4, space="PSUM") as pool_p3:
                for dt in range(D // 128):
                    ps3 = [
                        pool_p3.tile([128, 512], f32, name="ps3", tag="mm512")
                        for _ in range(2)
                    ]
                    for ct in range(NCT):
                        for jc in range(2):
                            nc.tensor.matmul(
                                ps3[jc][:],
                                wp_full[ct][:, dt * 128 : (dt + 1) * 128],
                                ao_heads[ct][:, jc * 512 : (jc + 1) * 512],
                                start=(ct == 0),
                                stop=(ct == NCT - 1),
                            )
                    for jc in range(2):
                        o_t = pool_out.tile([128, 512], f32, tag="outp")
                        nc.vector.tensor_scalar_add(
                            o_t[:], ps3[jc][:], bproj_sb[:, dt : dt + 1]
                        )
                        nc.sync.dma_start(
                            outT[dt * 128 : (dt + 1) * 128, jc * 512 : (jc + 1) * 512],
                            o_t[:],
                        )

            pool_qa_cm.__exit__(None, None, None)

    nc.compile()
    return nc


def _get_nc():
    if "nc" not in _CACHE:
        _CACHE["nc"] = _build()
    return _CACHE["nc"]


def kernel(x, w_qkv, b_qkv, w_proj, b_proj, _trace=False, _trace_kwargs=None):
    from concourse.bass_utils import run_bass_kernel_spmd

    x = np.asarray(x, dtype=np.float32)
    w_qkv = np.asarray(w_qkv, dtype=np.float32)
    b_qkv = np.asarray(b_qkv, dtype=np.float32)
    w_proj = np.asarray(w_proj, dtype=np.float32)
    b_proj = np.asarray(b_proj, dtype=np.float32)

    w_qkv16 = np.ascontiguousarray(w_qkv.astype(np.float16))
    w_proj16 = np.ascontiguousarray(w_proj.astype(np.float16))
    b_v16 = np.ascontiguousarray(b_qkv[2 * D : 3 * D].astype(np.float16))

    # packed constants: causal masks (keep where k<=q) for the two diagonal
    # k-tiles, a ones column, and a ones row
    consts = np.zeros((128, 2 * QC + 129), dtype=np.float16)
    p = np.arange(128)[:, None]
    f = np.arange(QC)[None, :]
    consts[:, 0:QC] = f >= p
    consts[:, QC : 2 * QC] = f >= p + 128
    consts[:, 2 * QC : 2 * QC + 129] = 1.0

    nc = _get_nc()
    in_maps = []
    for i in range(N_CORES):
        in_maps.append(
            {
                "xT": np.ascontiguousarray(x[i].T.astype(np.float16)),
                "w_qkv": w_qkv16,
                "b_qkv": b_qkv,
                "b_v": b_v16,
                "w_proj": w_proj16,
                "b_proj": b_proj,
                "consts": consts,
            }
        )
    res = run_bass_kernel_spmd(
        nc,
        in_maps,
        list(range(N_CORES)),
        trace=_trace,
        **(_trace_kwargs or {}),
    )
    y = np.stack(
        [np.ascontiguousarray(res.results[i]["outT"].T) for i in range(N_CORES)]
    )
    if _trace:
        _CACHE["last_result"] = res
    return y
